# revision 29
# baseline (speedup 1.0000x reference)
"""Trainium2 Bass kernel for 3-layer GAT + pooling readout (nn_GNN_7653631722064).

8-core SPMD, v3: core k owns dst nodes [1250k,1250k+1250) as 10 slots of 125.
Per layer: own-slot h-matmul (Wext=[W|ws|wd] emits es/ed in the same pass),
grouped per-slot AllGathers into a Shared [10000,1152]bf16 table (h bf16 +
es f32pairs, group/core/slot row layout), ED per chunk via one-hot S^T
matmul against local ed, ONE merged dma_gather per piece (h+es rows,
elem 1152), alpha=lrelu(es[src]+ED), ex=exp(alpha), M*=ex (6 heads on
vector, 2 on scalar), S matmuls (128-col padded stationary, 2x512-col rhs)
accumulate per-slot PSUM out + den; evict normalizes by 1/den, tree-adds
heads, relu+BN affine. Next layer's h-matmul + AllGather + ED are emitted
inside the piece loop right after each slot's evict so collectives overlap
the remaining aggregation. Readout local per core.
"""
import sys

sys.path.insert(0, "/opt/trn_rl_repo")

import numpy as np
import ml_dtypes

import concourse.bass as bass
import concourse.tile as tile
from concourse import bacc, mybir
from concourse.bass_utils import run_bass_kernel_spmd

BF16 = mybir.dt.bfloat16
FP8 = mybir.dt.float8e4
F32 = mybir.dt.float32
I16 = mybir.dt.int16
AF = mybir.ActivationFunctionType
OP = mybir.AluOpType

N, E, IN, H, C, G = 10000, 120000, 256, 8, 128, 64
NCORES = 8
TILE = 125
SLOTS = 10
PER_CORE = TILE * SLOTS
ROWB = 1280
EPS = 1e-5
SLOPE = 0.2
AG_GROUPS = [[0, 1, 2, 3], [4, 5, 6, 7], [8, 9]]
AG_GROUPS0 = AG_GROUPS
N_SCALAR_HEADS = 2


def _bf(a):
    return np.asarray(a, dtype=ml_dtypes.bfloat16)


def _f8(a):
    return np.asarray(a, dtype=ml_dtypes.float8_e4m3)


def build_nc(nct, chunk_lim, g0, cnt):
    nchunk = sum(nct)
    slot_c0 = np.concatenate([[0], np.cumsum(nct)]).astype(int)
    nc = bacc.Bacc(None, target_bir_lowering=False, debug=False,
                   num_devices=NCORES, name="gat")

    xT_in = nc.dram_tensor("xT", [128, 2, PER_CORE], BF16, kind="ExternalInput")
    w1_in = nc.dram_tensor("w1", [128, 2, 1040], BF16, kind="ExternalInput")
    w2_in = nc.dram_tensor("w2", [128, 1040], BF16, kind="ExternalInput")
    w3_in = nc.dram_tensor("w3", [128, 1040], BF16, kind="ExternalInput")
    s_in = nc.dram_tensor("sfull", [128, nchunk, 128], FP8, kind="ExternalInput")
    st_in = nc.dram_tensor("stfull", [TILE, nchunk, 128], BF16, kind="ExternalInput")
    gi_in = nc.dram_tensor("gidx", [128, nchunk * 8], I16, kind="ExternalInput")
    bn_in = nc.dram_tensor("bn", [128, 9, 128], BF16, kind="ExternalInput")
    id_in = nc.dram_tensor("ident", [128, 128], BF16, kind="ExternalInput")
    xr_in = nc.dram_tensor("xrootT", [128, 2, 8], BF16, kind="ExternalInput")
    l0w_in = nc.dram_tensor("l0w", [128, 2, 128], BF16, kind="ExternalInput")
    lnw_in = nc.dram_tensor("lnw", [128, 2, 128], BF16, kind="ExternalInput")
    l1w_in = nc.dram_tensor("l1w", [128, 2, 1], BF16, kind="ExternalInput")
    l0b_in = nc.dram_tensor("l0b", [128, 1], F32, kind="ExternalInput")
    lnb_in = nc.dram_tensor("lnb", [128, 1], F32, kind="ExternalInput")
    l1b_in = nc.dram_tensor("l1b", [8, 1], F32, kind="ExternalInput")
    out_t = nc.dram_tensor("out", [8, 1], F32, kind="ExternalOutput")

    warm_in = nc.dram_tensor("warm_in", [8, 128], FP8, kind="Internal")
    warm_out = nc.dram_tensor("warm_out", [64, 128], FP8, kind="Internal",
                              addr_space="Shared")
    shard = [nc.dram_tensor(f"shard{l}", [PER_CORE, ROWB], FP8, kind="Internal")
             for l in range(3)]
    full = [nc.dram_tensor(f"full{l}", [N, ROWB], FP8, kind="Internal",
                           addr_space="Shared")
            for l in range(3)]

    # pieces: split at tier (src-row-bound) boundaries and at 8 chunks.
    # slots processed 3rd or later merge all tiers (their AGs landed long
    # before), halving the per-call gather overhead.
    slot_pieces = []
    for s in range(SLOTS):
        ps = []
        c0 = int(slot_c0[s])
        end = int(slot_c0[s]) + nct[s]
        while c0 < end:
            if s >= 3:
                lim = N
                n = min(8, end - c0)
            else:
                lim = chunk_lim[c0]
                n = 1
                while (c0 + n < end and n < 8 and chunk_lim[c0 + n] == lim):
                    n += 1
            ps.append((c0, n, lim))
            c0 += n
        slot_pieces.append(ps)

    with tile.TileContext(nc) as tc:
        with (
            tc.tile_pool(name="persist", bufs=1) as pp,
            tc.tile_pool(name="work", bufs=2) as wp,
            tc.tile_pool(name="mbuf", bufs=8) as mp,
            tc.tile_pool(name="small", bufs=6) as sp,
            tc.tile_pool(name="pout", bufs=2, space="PSUM") as pout,
            tc.tile_pool(name="pden", bufs=2, space="PSUM") as pden,
            tc.tile_pool(name="pscr", bufs=2, space="PSUM") as pscr,
        ):
            XT = pp.tile([128, 2, PER_CORE], BF16)
            W1 = pp.tile([128, 2, 1040], BF16)
            W2 = pp.tile([128, 1040], BF16)
            W3 = pp.tile([128, 1040], BF16)
            S = pp.tile([128, nchunk, 128], FP8)
            ST = pp.tile([TILE, nchunk, 128], BF16)
            GI = pp.tile([128, nchunk * 8], I16)
            BN = pp.tile([128, 9, 128], BF16)
            IDT = pp.tile([128, 128], BF16)
            XR = pp.tile([128, 2, 8], BF16)
            L0W = pp.tile([128, 2, 128], BF16)
            LNW = pp.tile([128, 2, 128], BF16)
            L1W = pp.tile([128, 2, 1], BF16)
            L0B = pp.tile([128, 1], F32)
            LNB = pp.tile([128, 1], F32)
            L1B = pp.tile([8, 1], F32)
            for t, src in [(XT, xT_in), (W1, w1_in), (IDT, id_in), (BN, bn_in),
                           (W2, w2_in), (W3, w3_in), (XR, xr_in), (L0W, l0w_in),
                           (LNW, lnw_in), (L1W, l1w_in), (L0B, l0b_in),
                           (LNB, lnb_in), (L1B, l1b_in)]:
                nc.sync.dma_start(t[:], src[:])

            wtile = sp.tile([8, 128], FP8, tag="warm")
            nc.vector.memset(wtile[:], 0.0)
            nc.sync.dma_start(warm_in[:], wtile[:])
            nc.gpsimd.collective_compute(
                "AllGather", OP.bypass,
                replica_groups=[list(range(NCORES))],
                ins=[warm_in[:].opt()], outs=[warm_out[:].opt()])

            acts = {}
            esd = {}
            EDS = {}

            def lhs_for(l, ki, s):
                if l == 0:
                    return XT[:, ki, s * TILE:(s + 1) * TILE]
                return acts[(l - 1, s)][:]

            def tile_work(l, s):
                """h-matmul for layer l slot s + shard write + AG trigger."""
                wt = [W1, W2, W3][l]
                kc = 2 if l == 0 else 1

                def wslice(ki, off, w):
                    return wt[:, ki, off:off + w] if l == 0 else wt[:, off:off + w]

                hrow = wp.tile([TILE, 1024], FP8, tag="hrow")
                for off in (0, 512):
                    ph = pscr.tile([TILE, 512], F32, tag="scr")
                    for ki in range(kc):
                        nc.tensor.matmul(ph[:], lhs_for(l, ki, s),
                                         wslice(ki, off, 512),
                                         start=(ki == 0), stop=(ki == kc - 1))
                    nc.scalar.activation(hrow[:, off:off + 512], ph[:], AF.Copy)
                pe = pscr.tile([TILE, 512], F32, tag="scr")
                for ki in range(kc):
                    nc.tensor.matmul(pe[:, 0:16], lhs_for(l, ki, s),
                                     wslice(ki, 1024, 16),
                                     start=(ki == 0), stop=(ki == kc - 1))
                esdt = wp.tile([TILE, 16], F32, tag=f"esd{s}")
                nc.scalar.activation(esdt[:], pe[:, 0:16], AF.Copy)
                edb = wp.tile([TILE, 8], BF16, tag=f"edb{s}")
                nc.scalar.activation(edb[:], pe[:, 8:16], AF.Copy)
                esd[(l, s)] = edb
                nc.sync.dma_start(shard[l][s * TILE:(s + 1) * TILE, 0:1024],
                                  hrow[:])
                nc.sync.dma_start(shard[l][s * TILE:(s + 1) * TILE, 1024:1056],
                                  esdt[:, 0:8].bitcast(FP8))


            def emit_ag(l, grp):
                gl = len(grp)
                r0 = grp[0] * TILE
                f0 = grp[0] * TILE * NCORES
                nc.gpsimd.collective_compute(
                    "AllGather", OP.bypass,
                    replica_groups=[list(range(NCORES))],
                    ins=[shard[l][r0:r0 + gl * TILE, :].opt()],
                    outs=[full[l][f0:f0 + gl * TILE * NCORES, :].opt()])

            def ed_work(l, s):
                """per-chunk ED for layer l slot s via one-hot S^T matmul."""
                if (l, "eds") not in EDS:
                    edst = wp.tile([128, nchunk, 8], F32, tag="EDS",
                                   name=f"EDS{l}")
                    EDS[(l, "eds")] = edst
                eds = EDS[(l, "eds")]
                nct_s = nct[s]
                edp = pscr.tile([128, 512], F32, tag="scr")
                for ci in range(nct_s):
                    c = slot_c0[s] + ci
                    nc.tensor.matmul(edp[:, ci * 8:(ci + 1) * 8],
                                     ST[:, c, :], esd[(l, s)][:],
                                     start=True, stop=True)
                nc.vector.tensor_copy(eds[:, slot_c0[s]:slot_c0[s] + nct_s, :],
                                      edp[:, 0:nct_s * 8])

            psums = {}

            def graph_reduce(g):
                gm = sp.tile([128, 1], F32, tag="gm")
                nc.vector.tensor_reduce(gm[:], a3[:, g0[g]:g0[g + 1]],
                                        mybir.AxisListType.X, OP.max)
                nc.vector.tensor_copy(gmpb[:, g:g + 1], gm[:])
                ga = sp.tile([128, 1], F32, tag="ga")
                nc.vector.tensor_reduce(ga[:], a3[:, g0[g]:g0[g + 1]],
                                        mybir.AxisListType.X, OP.add)
                nc.vector.tensor_scalar_mul(ga[:], ga[:], 1.0 / cnt[g])
                nc.vector.tensor_copy(gapb[:, g:g + 1], ga[:])

            def piece_work(l, s, lims):
                if (l, s) not in psums:
                    outp = pout.tile([128, 1024], F32, tag="outps",
                                     name=f"o{l}_{s}")
                    denp = pden.tile([128, 8], F32, tag="denps",
                                     name=f"d{l}_{s}")
                    psums[(l, s)] = (outp, denp)
                outp, denp = psums[(l, s)]
                eds = EDS[(l, "eds")]
                cfirst, clast = slot_c0[s], slot_c0[s] + nct[s] - 1
                pieces = [p for p in slot_pieces[s] if p[2] in lims]
                for (c0, nch, lim) in pieces:
                    ne = nch * 128
                    M = mp.tile([128, 8, ROWB], FP8, tag="M")
                    nc.gpsimd.dma_gather(
                        out_ap=M[:, 0:nch, :], in_ap=full[l][0:lim],
                        idxs_ap=GI[:, c0 * 8:c0 * 8 + nch * 8],
                        num_idxs=ne, num_idxs_reg=ne,
                        elem_size=ROWB, elem_step=ROWB)
                    alpha = sp.tile([128, 8, 8], F32, tag="alpha")
                    nc.gpsimd.tensor_tensor(
                        alpha[:, 0:nch, :],
                        M[:].bitcast(F32)[:, 0:nch, 256:264],
                        eds[:, c0:c0 + nch, :], OP.add)
                    lr = sp.tile([128, 8, 8], F32, tag="lr")
                    nc.vector.tensor_scalar_mul(lr[:, 0:nch, :],
                                                alpha[:, 0:nch, :], SLOPE)
                    nc.vector.tensor_tensor(lr[:, 0:nch, :], lr[:, 0:nch, :],
                                            alpha[:, 0:nch, :], OP.max)
                    exf = sp.tile([128, 8, 8], F32, tag="exf")
                    nc.scalar.activation(exf[:, 0:nch, :], lr[:, 0:nch, :],
                                         AF.Exp)
                    exq = sp.tile([128, 8, 8], FP8, tag="exq")
                    nc.vector.tensor_copy(exq[:, 0:nch, :], exf[:, 0:nch, :])
                    for h in range(H - N_SCALAR_HEADS):
                        mh = M[:, 0:nch, h * C:(h + 1) * C]
                        exv = exq[:, 0:nch, h:h + 1].broadcast_to((128, nch, C))
                        nc.vector.tensor_tensor(mh, mh, exv, OP.mult)
                    for h in range(H - N_SCALAR_HEADS, H):
                        for c8 in range(nch):
                            mh = M[:, c8, h * C:(h + 1) * C]
                            nc.scalar.activation(mh, mh, AF.Copy,
                                                 scale=exf[:, c8, h:h + 1])
                    for c8 in range(nch):
                        c = c0 + c8
                        st_, sp_ = (c == cfirst), (c == clast)
                        nc.tensor.matmul(denp[:], S[:, c, :], exq[:, c8, :],
                                         start=st_, stop=sp_)
                        nc.tensor.matmul(outp[:, 0:512], S[:, c, :],
                                         M[:, c8, 0:512], start=st_, stop=sp_)
                        nc.tensor.matmul(outp[:, 512:1024], S[:, c, :],
                                         M[:, c8, 512:1024], start=st_, stop=sp_)

            def evict_work(l, s):
                outp, denp = psums[(l, s)]
                # ---- evict slot s ---------------------------------------
                rden = sp.tile([TILE, 8], F32, tag="rden")
                nc.vector.reciprocal(rden[:], denp[0:TILE, :])
                tsc = wp.tile([TILE, 1024], BF16, tag="tsc")
                for h in range(4):
                    nc.vector.tensor_scalar_mul(tsc[:, h * C:(h + 1) * C],
                                                outp[0:TILE, h * C:(h + 1) * C],
                                                rden[:, h:h + 1])
                for h in range(4, H):
                    nc.scalar.activation(tsc[:, h * C:(h + 1) * C],
                                         outp[0:TILE, h * C:(h + 1) * C],
                                         AF.Copy, scale=rden[:, h:h + 1])
                nc.vector.tensor_tensor(tsc[:, 0:512], tsc[:, 0:512],
                                        tsc[:, 512:1024], OP.add)
                nc.vector.tensor_tensor(tsc[:, 0:256], tsc[:, 0:256],
                                        tsc[:, 256:512], OP.add)
                nc.vector.tensor_tensor(tsc[:, 0:128], tsc[:, 0:128],
                                        tsc[:, 128:256], OP.add)
                summ = tsc[:, 0:128]
                nc.vector.tensor_tensor(summ, summ, BN[0:TILE, 3 * l, :], OP.add)
                nc.vector.tensor_scalar_max(summ, summ, 0.0)
                nc.vector.tensor_tensor(summ, summ, BN[0:TILE, 3 * l + 1, :],
                                        OP.mult)
                act = wp.tile([TILE, 128], BF16, tag=f"act{s}")
                nc.vector.tensor_tensor(act[:], summ, BN[0:TILE, 3 * l + 2, :],
                                        OP.add)
                ptp = pscr.tile([128, TILE], BF16, tag="scr")
                nc.tensor.transpose(ptp[:], act[:], IDT[0:TILE, 0:TILE])
                if l < 2:
                    at = wp.tile([128, TILE], BF16, tag=f"actT{s}")
                    nc.vector.tensor_copy(at[:], ptp[:])
                    acts[(l, s)] = at
                else:
                    nc.vector.tensor_copy(a3[:, s * TILE:(s + 1) * TILE], ptp[:])
                    for g in range(8):
                        lastslot = (g0[g + 1] - 1) // TILE
                        if lastslot == s:
                            graph_reduce(g)

            # ---- layer 0 tile phase + ED --------------------------------
            for s in range(SLOTS):
                tile_work(0, s)
            # big graph tables load after the shard writes are queued
            for t, src in [(ST, st_in), (GI, gi_in), (S, s_in)]:
                nc.sync.dma_start(t[:], src[:])
            emit_ag(0, [0, 1, 2, 3])
            emit_ag(0, [4, 5, 6, 7])
            emit_ag(0, [8, 9])
            for s in range(SLOTS):
                ed_work(0, s)
            a3 = pp.tile([128, PER_CORE], BF16)
            gmpb = pp.tile([128, 8], BF16)
            gapb = pp.tile([128, 8], BF16)
            # ---- layers -------------------------------------------------
            # Explicit emission schedule: slots 0/1 interleave their tier
            # pieces at the layer boundary so the tail AllGathers (emitted
            # after the first t0 gathers) never block the GpSimd queue; t2
            # pieces + evict lag one slot behind; each next-layer AllGather
            # trigger is emitted ~2 slots after its producing tile so its
            # input wait is zero.
            T0, T1, T2, ALL = (4000,), (8000,), (N,), (4000, 8000, N)

            def nxt(l, s):
                if l < 2:
                    tile_work(l + 1, s)
                    ed_work(l + 1, s)

            for l in range(3):
                piece_work(l, 0, T0)
                piece_work(l, 1, T0)
                if l > 0:
                    emit_ag(l, [4, 5, 6, 7])
                    emit_ag(l, [8, 9])
                piece_work(l, 0, T1)
                piece_work(l, 1, T1)
                piece_work(l, 0, T2)
                evict_work(l, 0)
                nxt(l, 0)
                piece_work(l, 2, (4000, 8000))
                piece_work(l, 1, T2)
                evict_work(l, 1)
                nxt(l, 1)
                piece_work(l, 3, ALL)
                piece_work(l, 2, T2)
                evict_work(l, 2)
                nxt(l, 2)
                for s in range(4, SLOTS):
                    piece_work(l, s, ALL)
                    evict_work(l, s - 1)
                    nxt(l, s - 1)
                    if l < 2 and s == 7:
                        emit_ag(l + 1, [0, 1, 2, 3])
                evict_work(l, SLOTS - 1)
                nxt(l, SLOTS - 1)

            # ---- readout ------------------------------------------------
            phg = pscr.tile([128, 8], F32, tag="scr")
            nc.tensor.matmul(phg[:], L0W[:, 0, :], gmpb[:], start=True, stop=False)
            nc.tensor.matmul(phg[:], L0W[:, 1, :], gapb[:], start=False, stop=True)
            hg = pp.tile([128, 8], BF16)
            nc.scalar.activation(hg[:], phg[:], AF.Relu, bias=L0B[:])
            pnw = pscr.tile([128, 8], F32, tag="scr")
            nc.tensor.matmul(pnw[:], LNW[:, 0, :], XR[:, 0, :], start=True, stop=False)
            nc.tensor.matmul(pnw[:], LNW[:, 1, :], XR[:, 1, :], start=False, stop=True)
            nw = pp.tile([128, 8], BF16)
            nc.scalar.activation(nw[:], pnw[:], AF.Relu, bias=LNB[:])
            pfin = pscr.tile([8, 1], F32, tag="scr")
            nc.tensor.matmul(pfin[:], hg[:], L1W[:, 0, :], start=True, stop=False)
            nc.tensor.matmul(pfin[:], nw[:], L1W[:, 1, :], start=False, stop=True)
            fin = pp.tile([8, 1], F32)
            nc.scalar.activation(fin[:], pfin[:], AF.Sigmoid, bias=L1B[:])
            nc.sync.dma_start(out_t[:], fin[:])
    nc.compile()
    return nc


def kernel(x, edge_index, batch,
           W1, as1, ad1, b1, g1, bb1, m1, v1,
           W2, as2, ad2, b2, g2, bb2, m2, v2,
           W3, as3, ad3, b3, g3, bb3, m3, v3,
           lnW, lnb, l0W, l0b, l1W, l1b):
    x = np.asarray(x, np.float32)
    edge_index = np.asarray(edge_index)
    batch = np.asarray(batch)
    Ws = [np.asarray(w, np.float64) for w in (W1, W2, W3)]
    ass = [np.asarray(a, np.float64) for a in (as1, as2, as3)]
    ads = [np.asarray(a, np.float64) for a in (ad1, ad2, ad3)]
    bs = [np.asarray(a, np.float32) for a in (b1, b2, b3)]
    gs = [np.asarray(a, np.float32) for a in (g1, g2, g3)]
    bbs = [np.asarray(a, np.float32) for a in (bb1, bb2, bb3)]
    ms = [np.asarray(a, np.float32) for a in (m1, m2, m3)]
    vs = [np.asarray(a, np.float32) for a in (v1, v2, v3)]

    src = np.concatenate([edge_index[0], np.arange(N)]).astype(np.int64)
    dst = np.concatenate([edge_index[1], np.arange(N)]).astype(np.int64)
    tile_of = dst // TILE
    order = np.argsort(tile_of, kind="stable")
    src, dst, tile_of = src[order], dst[order], tile_of[order]
    bounds = np.searchsorted(tile_of, np.arange(81))
    cnts_ks = (bounds[1:] - bounds[:-1]).reshape(NCORES, SLOTS)
    nct = [int(np.ceil(cnts_ks[:, s] / 128).max()) for s in range(SLOTS)]
    nchunk = sum(nct)
    slot_c0 = np.concatenate([[0], np.cumsum(nct)]).astype(int)

    # grouped full-table row layout (same groups for every layer; layer 0
    # uses one big AG but the row layout only depends on AG_GROUPS for
    # consistency across layers -> use AG_GROUPS layout for all).
    full_row = np.zeros(N, np.int64)
    base = 0
    for grp in AG_GROUPS:
        gl = len(grp)
        for k in range(NCORES):
            for si, s in enumerate(grp):
                rows = np.arange(TILE) + k * PER_CORE + s * TILE
                full_row[rows] = base + k * TILE * gl + si * TILE + np.arange(TILE)
        base += gl * TILE * NCORES
    assert base == N

    gsrc = np.zeros((NCORES, nchunk * 128), np.int16)
    dloc = np.full((NCORES, nchunk * 128), -1, np.int64)
    tier_cnt = np.zeros((NCORES, SLOTS, 2), np.int64)
    for t in range(80):
        k, s = t // SLOTS, t % SLOTS
        lo, hi = bounds[t], bounds[t + 1]
        fr = full_row[src[lo:hi]]
        tier = (fr >= 4000).astype(np.int64) + (fr >= 8000)
        tord = np.argsort(tier, kind="stable")
        fr, dl_ = fr[tord], (dst[lo:hi] - t * TILE)[tord]
        tier_cnt[k, s, 0] = int((tier == 0).sum())
        tier_cnt[k, s, 1] = int((tier <= 1).sum())
        o = slot_c0[s] * 128
        gsrc[k, o:o + hi - lo] = fr
        dloc[k, o:o + hi - lo] = dl_
    # per-chunk src-row bound = min tier across cores (pad idx 0 is tier 0)
    chunk_lim = np.full(nchunk, N, np.int64)
    for s in range(SLOTS):
        for ci in range(nct[s]):
            e_end = (ci + 1) * 128
            if all(e_end <= tier_cnt[k, s, 0] for k in range(NCORES)):
                chunk_lim[slot_c0[s] + ci] = 4000
            elif all(e_end <= tier_cnt[k, s, 1] for k in range(NCORES)):
                chunk_lim[slot_c0[s] + ci] = 8000
    dl = dloc.reshape(NCORES, nchunk, 128)
    S_all = dl[:, :, :, None] == np.arange(128)[None, None, None, :]
    S_t = _f8(S_all.transpose(0, 2, 1, 3))   # [NCORES, 128e, nchunk, 128d]
    ST_t = _bf(S_all.transpose(0, 3, 1, 2)[:, 0:TILE])  # [NCORES, 125d, nchunk, 128e]
    gi = gsrc.reshape(NCORES, nchunk * 8, 16).transpose(0, 2, 1)
    gi = np.ascontiguousarray(np.tile(gi, (1, 8, 1)))

    wexts = []
    for li in range(3):
        W = Ws[li]
        ws = np.stack([W[:, h * C:(h + 1) * C] @ ass[li][h] for h in range(H)], 1)
        wd = np.stack([W[:, h * C:(h + 1) * C] @ ads[li][h] for h in range(H)], 1)
        wexts.append(np.concatenate([W, ws, wd], axis=1))
    w1 = _bf(wexts[0].reshape(2, 128, 1040).transpose(1, 0, 2))
    w2 = _bf(wexts[1][0:128])
    w3 = _bf(wexts[2][0:128])
    bn = np.zeros((128, 9, 128), np.float32)  # cast to bf16 below
    for li in range(3):
        r = 1.0 / np.sqrt(vs[li] + EPS)
        bn[:, 3 * li + 0] = 8.0 * bs[li]
        bn[:, 3 * li + 1] = gs[li] * r / 8.0
        bn[:, 3 * li + 2] = bbs[li] - ms[li] * gs[li] * r

    bnds = np.searchsorted(batch, np.arange(G + 1))
    assert bnds[G] == N
    for k in range(1, NCORES):
        assert bnds[8 * k] == PER_CORE * k, "graphs must align to cores"
    g0 = [int(bnds[g]) for g in range(9)]
    cnt = [float(bnds[g + 1] - bnds[g]) for g in range(8)]
    for k in range(1, NCORES):
        for g in range(9):
            assert int(bnds[8 * k + g]) - PER_CORE * k == g0[g]
    root = bnds[:G]
    xrT = np.asarray(x, np.float64)[root].T.reshape(2, 128, G)

    l0w = _bf(np.asarray(l0W).reshape(2, 128, 128).transpose(1, 0, 2))
    lnw = _bf(np.asarray(lnW).reshape(2, 128, 128).transpose(1, 0, 2))
    l1w = _bf(np.asarray(l1W).reshape(2, 128, 1).transpose(1, 0, 2))

    nc = build_nc(nct, [int(v) for v in chunk_lim], g0, cnt)

    x64 = np.asarray(x, np.float64)
    in_maps = []
    for k in range(NCORES):
        xk = x64[k * PER_CORE:(k + 1) * PER_CORE]     # [1250, 256]
        xTk = _bf(np.ascontiguousarray(
            xk.T.reshape(2, 128, PER_CORE).transpose(1, 0, 2)))
        in_maps.append(dict(
            xT=xTk,
            w1=w1, w2=w2, w3=w3,
            sfull=np.ascontiguousarray(S_t[k]),
            stfull=np.ascontiguousarray(ST_t[k]),
            gidx=gi[k],
            bn=_bf(bn), ident=_bf(np.eye(128)),
            xrootT=_bf(np.ascontiguousarray(
                xrT[:, :, 8 * k:8 * k + 8].transpose(1, 0, 2))),
            l0w=l0w, lnw=lnw, l1w=l1w,
            l0b=np.asarray(l0b, np.float32).reshape(128, 1),
            lnb=np.asarray(lnb, np.float32).reshape(128, 1),
            l1b=np.broadcast_to(np.asarray(l1b, np.float32), (8, 1)).copy(),
        ))
    global LAST_RESULT, LAST_NC, LAST_INMAPS
    LAST_NC, LAST_INMAPS = nc, in_maps
    res = run_bass_kernel_spmd(nc, in_maps, core_ids=list(range(NCORES)))
    LAST_RESULT = res
    out = np.concatenate([res.results[k]["out"] for k in range(NCORES)], 0)
    return out.astype(np.float32)


# revision 30
# speedup vs baseline: 2.0596x; 2.0596x over previous
"""Trainium2 Bass kernel for 3-layer GAT + pooling readout (nn_GNN_7653631722064).

8-core SPMD, v3: core k owns dst nodes [1250k,1250k+1250) as 10 slots of 125.
Per layer: own-slot h-matmul (Wext=[W|ws|wd] emits es/ed in the same pass),
grouped per-slot AllGathers into a Shared [10000,1152]bf16 table (h bf16 +
es f32pairs, group/core/slot row layout), ED per chunk via one-hot S^T
matmul against local ed, ONE merged dma_gather per piece (h+es rows,
elem 1152), alpha=lrelu(es[src]+ED), ex=exp(alpha), M*=ex (6 heads on
vector, 2 on scalar), S matmuls (128-col padded stationary, 2x512-col rhs)
accumulate per-slot PSUM out + den; evict normalizes by 1/den, tree-adds
heads, relu+BN affine. Next layer's h-matmul + AllGather + ED are emitted
inside the piece loop right after each slot's evict so collectives overlap
the remaining aggregation. Readout local per core.
"""
import sys

sys.path.insert(0, "/opt/trn_rl_repo")

import numpy as np
import ml_dtypes

import concourse.bass as bass
import concourse.tile as tile
from concourse import bacc, mybir
from concourse.bass_utils import run_bass_kernel_spmd

BF16 = mybir.dt.bfloat16
FP8 = mybir.dt.float8e4
F32 = mybir.dt.float32
I16 = mybir.dt.int16
AF = mybir.ActivationFunctionType
OP = mybir.AluOpType

N, E, IN, H, C, G = 10000, 120000, 256, 8, 128, 64
NCORES = 8
TILE = 125
SLOTS = 10
PER_CORE = TILE * SLOTS
ROWB = 1280
EPS = 1e-5
SLOPE = 0.2
AG_GROUPS = [[0, 1, 2, 3], [4, 5, 6, 7], [8, 9]]
AG_GROUPS0 = AG_GROUPS
N_SCALAR_HEADS = 2


def _bf(a):
    return np.asarray(a, dtype=ml_dtypes.bfloat16)


def _f8(a):
    return np.asarray(a, dtype=ml_dtypes.float8_e4m3)


def build_nc(nct, chunk_lim, g0, cnt):
    nchunk = sum(nct)
    slot_c0 = np.concatenate([[0], np.cumsum(nct)]).astype(int)
    nc = bacc.Bacc(None, target_bir_lowering=False, debug=False,
                   num_devices=NCORES, name="gat")

    xT_in = nc.dram_tensor("xT", [128, 2, PER_CORE], BF16, kind="ExternalInput")
    w1_in = nc.dram_tensor("w1", [128, 2, 1040], BF16, kind="ExternalInput")
    w2_in = nc.dram_tensor("w2", [128, 1040], BF16, kind="ExternalInput")
    w3_in = nc.dram_tensor("w3", [128, 1040], BF16, kind="ExternalInput")
    s_in = nc.dram_tensor("sfull", [128, nchunk, 128], FP8, kind="ExternalInput")
    st_in = nc.dram_tensor("stfull", [TILE, nchunk, 128], BF16, kind="ExternalInput")
    gi_in = nc.dram_tensor("gidx", [128, nchunk * 8], I16, kind="ExternalInput")
    bn_in = nc.dram_tensor("bn", [128, 9, 128], BF16, kind="ExternalInput")
    id_in = nc.dram_tensor("ident", [128, 128], BF16, kind="ExternalInput")
    xr_in = nc.dram_tensor("xrootT", [128, 2, 8], BF16, kind="ExternalInput")
    l0w_in = nc.dram_tensor("l0w", [128, 2, 128], BF16, kind="ExternalInput")
    lnw_in = nc.dram_tensor("lnw", [128, 2, 128], BF16, kind="ExternalInput")
    l1w_in = nc.dram_tensor("l1w", [128, 2, 1], BF16, kind="ExternalInput")
    l0b_in = nc.dram_tensor("l0b", [128, 1], F32, kind="ExternalInput")
    lnb_in = nc.dram_tensor("lnb", [128, 1], F32, kind="ExternalInput")
    l1b_in = nc.dram_tensor("l1b", [8, 1], F32, kind="ExternalInput")
    out_t = nc.dram_tensor("out", [8, 1], F32, kind="ExternalOutput")

    warm_in = nc.dram_tensor("warm_in", [8, 128], FP8, kind="Internal")
    warm_out = nc.dram_tensor("warm_out", [64, 128], FP8, kind="Internal",
                              addr_space="Shared")
    shard = [nc.dram_tensor(f"shard{l}", [PER_CORE, ROWB], FP8, kind="Internal")
             for l in range(3)]
    full = [nc.dram_tensor(f"full{l}", [N, ROWB], FP8, kind="Internal",
                           addr_space="Shared")
            for l in range(3)]

    # pieces: split at tier (src-row-bound) boundaries and at 8 chunks.
    # slots processed 3rd or later merge all tiers (their AGs landed long
    # before), halving the per-call gather overhead.
    slot_pieces = []
    for s in range(SLOTS):
        ps = []
        c0 = int(slot_c0[s])
        end = int(slot_c0[s]) + nct[s]
        while c0 < end:
            if s >= 3:
                lim = N
                n = min(8, end - c0)
            else:
                lim = chunk_lim[c0]
                n = 1
                while (c0 + n < end and n < 8 and chunk_lim[c0 + n] == lim):
                    n += 1
            ps.append((c0, n, lim))
            c0 += n
        slot_pieces.append(ps)

    with tile.TileContext(nc) as tc:
        with (
            tc.tile_pool(name="persist", bufs=1) as pp,
            tc.tile_pool(name="work", bufs=2) as wp,
            tc.tile_pool(name="mbuf", bufs=8) as mp,
            tc.tile_pool(name="small", bufs=6) as sp,
            tc.tile_pool(name="pout", bufs=2, space="PSUM") as pout,
            tc.tile_pool(name="pden", bufs=2, space="PSUM") as pden,
            tc.tile_pool(name="pscr", bufs=2, space="PSUM") as pscr,
        ):
            XT = pp.tile([128, 2, PER_CORE], BF16)
            W1 = pp.tile([128, 2, 1040], BF16)
            W2 = pp.tile([128, 1040], BF16)
            W3 = pp.tile([128, 1040], BF16)
            S = pp.tile([128, nchunk, 128], FP8)
            ST = pp.tile([TILE, nchunk, 128], BF16)
            GI = pp.tile([128, nchunk * 8], I16)
            BN = pp.tile([128, 9, 128], BF16)
            IDT = pp.tile([128, 128], BF16)
            XR = pp.tile([128, 2, 8], BF16)
            L0W = pp.tile([128, 2, 128], BF16)
            LNW = pp.tile([128, 2, 128], BF16)
            L1W = pp.tile([128, 2, 1], BF16)
            L0B = pp.tile([128, 1], F32)
            LNB = pp.tile([128, 1], F32)
            L1B = pp.tile([8, 1], F32)
            for t, src in [(XT, xT_in), (W1, w1_in), (IDT, id_in), (BN, bn_in),
                           (W2, w2_in), (W3, w3_in), (XR, xr_in), (L0W, l0w_in),
                           (LNW, lnw_in), (L1W, l1w_in), (L0B, l0b_in),
                           (LNB, lnb_in), (L1B, l1b_in)]:
                nc.sync.dma_start(t[:], src[:])

            wtile = sp.tile([8, 128], FP8, tag="warm")
            nc.vector.memset(wtile[:], 0.0)
            nc.sync.dma_start(warm_in[:], wtile[:])
            nc.gpsimd.collective_compute(
                "AllGather", OP.bypass,
                replica_groups=[list(range(NCORES))],
                ins=[warm_in[:].opt()], outs=[warm_out[:].opt()])

            acts = {}
            esd = {}
            EDS = {}

            def lhs_for(l, ki, s):
                if l == 0:
                    return XT[:, ki, s * TILE:(s + 1) * TILE]
                return acts[(l - 1, s)][:]

            def tile_work(l, s):
                """h-matmul for layer l slot s + shard write + AG trigger."""
                wt = [W1, W2, W3][l]
                kc = 2 if l == 0 else 1

                def wslice(ki, off, w):
                    return wt[:, ki, off:off + w] if l == 0 else wt[:, off:off + w]

                hrow = wp.tile([TILE, 1024], FP8, tag="hrow")
                for off in (0, 512):
                    ph = pscr.tile([TILE, 512], F32, tag="scr")
                    for ki in range(kc):
                        nc.tensor.matmul(ph[:], lhs_for(l, ki, s),
                                         wslice(ki, off, 512),
                                         start=(ki == 0), stop=(ki == kc - 1))
                    nc.scalar.activation(hrow[:, off:off + 512], ph[:], AF.Copy)
                pe = pscr.tile([TILE, 512], F32, tag="scr")
                for ki in range(kc):
                    nc.tensor.matmul(pe[:, 0:16], lhs_for(l, ki, s),
                                     wslice(ki, 1024, 16),
                                     start=(ki == 0), stop=(ki == kc - 1))
                esdt = wp.tile([TILE, 16], F32, tag=f"esd{s}")
                nc.scalar.activation(esdt[:], pe[:, 0:16], AF.Copy)
                edb = wp.tile([TILE, 8], BF16, tag=f"edb{s}")
                nc.scalar.activation(edb[:], pe[:, 8:16], AF.Copy)
                esd[(l, s)] = edb
                nc.sync.dma_start(shard[l][s * TILE:(s + 1) * TILE, 0:1024],
                                  hrow[:])
                nc.sync.dma_start(shard[l][s * TILE:(s + 1) * TILE, 1024:1056],
                                  esdt[:, 0:8].bitcast(FP8))


            def emit_ag(l, grp):
                gl = len(grp)
                r0 = grp[0] * TILE
                f0 = grp[0] * TILE * NCORES
                nc.gpsimd.collective_compute(
                    "AllGather", OP.bypass,
                    replica_groups=[list(range(NCORES))],
                    ins=[shard[l][r0:r0 + gl * TILE, :].opt()],
                    outs=[full[l][f0:f0 + gl * TILE * NCORES, :].opt()])

            def ed_work(l, s):
                """per-chunk ED for layer l slot s via one-hot S^T matmul."""
                if (l, "eds") not in EDS:
                    edst = wp.tile([128, nchunk, 8], F32, tag="EDS",
                                   name=f"EDS{l}")
                    EDS[(l, "eds")] = edst
                eds = EDS[(l, "eds")]
                nct_s = nct[s]
                edp = pscr.tile([128, 512], F32, tag="scr")
                for ci in range(nct_s):
                    c = slot_c0[s] + ci
                    nc.tensor.matmul(edp[:, ci * 8:(ci + 1) * 8],
                                     ST[:, c, :], esd[(l, s)][:],
                                     start=True, stop=True)
                nc.vector.tensor_copy(eds[:, slot_c0[s]:slot_c0[s] + nct_s, :],
                                      edp[:, 0:nct_s * 8])

            psums = {}

            def graph_reduce(g):
                gm = sp.tile([128, 1], F32, tag="gm")
                nc.vector.tensor_reduce(gm[:], a3[:, g0[g]:g0[g + 1]],
                                        mybir.AxisListType.X, OP.max)
                nc.vector.tensor_copy(gmpb[:, g:g + 1], gm[:])
                ga = sp.tile([128, 1], F32, tag="ga")
                nc.vector.tensor_reduce(ga[:], a3[:, g0[g]:g0[g + 1]],
                                        mybir.AxisListType.X, OP.add)
                nc.vector.tensor_scalar_mul(ga[:], ga[:], 1.0 / cnt[g])
                nc.vector.tensor_copy(gapb[:, g:g + 1], ga[:])

            def piece_work(l, s, lims):
                if (l, s) not in psums:
                    outp = pout.tile([128, 1024], F32, tag="outps",
                                     name=f"o{l}_{s}")
                    denp = pden.tile([128, 8], F32, tag="denps",
                                     name=f"d{l}_{s}")
                    psums[(l, s)] = (outp, denp)
                outp, denp = psums[(l, s)]
                eds = EDS[(l, "eds")]
                cfirst, clast = slot_c0[s], slot_c0[s] + nct[s] - 1
                pieces = [p for p in slot_pieces[s] if p[2] in lims]
                for (c0, nch, lim) in pieces:
                    ne = nch * 128
                    M = mp.tile([128, 8, ROWB], FP8, tag="M")
                    nc.gpsimd.dma_gather(
                        out_ap=M[:, 0:nch, :], in_ap=full[l][0:lim],
                        idxs_ap=GI[:, c0 * 8:c0 * 8 + nch * 8],
                        num_idxs=ne, num_idxs_reg=ne,
                        elem_size=ROWB, elem_step=ROWB)
                    alpha = sp.tile([128, 8, 8], F32, tag="alpha")
                    nc.vector.tensor_tensor(
                        alpha[:, 0:nch, :],
                        M[:].bitcast(F32)[:, 0:nch, 256:264],
                        eds[:, c0:c0 + nch, :], OP.add)
                    lr = sp.tile([128, 8, 8], F32, tag="lr")
                    nc.vector.tensor_scalar_mul(lr[:, 0:nch, :],
                                                alpha[:, 0:nch, :], SLOPE)
                    nc.vector.tensor_tensor(lr[:, 0:nch, :], lr[:, 0:nch, :],
                                            alpha[:, 0:nch, :], OP.max)
                    exf = sp.tile([128, 8, 8], F32, tag="exf")
                    nc.scalar.activation(exf[:, 0:nch, :], lr[:, 0:nch, :],
                                         AF.Exp)
                    exq = sp.tile([128, 8, 8], FP8, tag="exq")
                    nc.vector.tensor_copy(exq[:, 0:nch, :], exf[:, 0:nch, :])
                    for h in range(H - N_SCALAR_HEADS):
                        mh = M[:, 0:nch, h * C:(h + 1) * C]
                        exv = exq[:, 0:nch, h:h + 1].broadcast_to((128, nch, C))
                        nc.vector.tensor_tensor(mh, mh, exv, OP.mult)
                    for h in range(H - N_SCALAR_HEADS, H):
                        for c8 in range(nch):
                            mh = M[:, c8, h * C:(h + 1) * C]
                            nc.scalar.activation(mh, mh, AF.Copy,
                                                 scale=exf[:, c8, h:h + 1])
                    for c8 in range(nch):
                        c = c0 + c8
                        st_, sp_ = (c == cfirst), (c == clast)
                        nc.tensor.matmul(denp[:], S[:, c, :], exq[:, c8, :],
                                         start=st_, stop=sp_)
                        nc.tensor.matmul(outp[:, 0:512], S[:, c, :],
                                         M[:, c8, 0:512], start=st_, stop=sp_)
                        nc.tensor.matmul(outp[:, 512:1024], S[:, c, :],
                                         M[:, c8, 512:1024], start=st_, stop=sp_)

            def evict_work(l, s):
                outp, denp = psums[(l, s)]
                # ---- evict slot s ---------------------------------------
                rden = sp.tile([TILE, 8], F32, tag="rden")
                nc.vector.reciprocal(rden[:], denp[0:TILE, :])
                tsc = wp.tile([TILE, 1024], BF16, tag="tsc")
                for h in range(4):
                    nc.vector.tensor_scalar_mul(tsc[:, h * C:(h + 1) * C],
                                                outp[0:TILE, h * C:(h + 1) * C],
                                                rden[:, h:h + 1])
                for h in range(4, H):
                    nc.scalar.activation(tsc[:, h * C:(h + 1) * C],
                                         outp[0:TILE, h * C:(h + 1) * C],
                                         AF.Copy, scale=rden[:, h:h + 1])
                nc.vector.tensor_tensor(tsc[:, 0:512], tsc[:, 0:512],
                                        tsc[:, 512:1024], OP.add)
                nc.vector.tensor_tensor(tsc[:, 0:256], tsc[:, 0:256],
                                        tsc[:, 256:512], OP.add)
                nc.vector.tensor_tensor(tsc[:, 0:128], tsc[:, 0:128],
                                        tsc[:, 128:256], OP.add)
                summ = tsc[:, 0:128]
                nc.vector.tensor_tensor(summ, summ, BN[0:TILE, 3 * l, :], OP.add)
                nc.vector.tensor_scalar_max(summ, summ, 0.0)
                nc.vector.tensor_tensor(summ, summ, BN[0:TILE, 3 * l + 1, :],
                                        OP.mult)
                act = wp.tile([TILE, 128], BF16, tag=f"act{s}")
                nc.vector.tensor_tensor(act[:], summ, BN[0:TILE, 3 * l + 2, :],
                                        OP.add)
                ptp = pscr.tile([128, TILE], BF16, tag="scr")
                nc.tensor.transpose(ptp[:], act[:], IDT[0:TILE, 0:TILE])
                if l < 2:
                    at = wp.tile([128, TILE], BF16, tag=f"actT{s}")
                    nc.vector.tensor_copy(at[:], ptp[:])
                    acts[(l, s)] = at
                else:
                    nc.vector.tensor_copy(a3[:, s * TILE:(s + 1) * TILE], ptp[:])
                    for g in range(8):
                        lastslot = (g0[g + 1] - 1) // TILE
                        if lastslot == s:
                            graph_reduce(g)

            # ---- layer 0 tile phase + ED --------------------------------
            for s in range(SLOTS):
                tile_work(0, s)
            # big graph tables load after the shard writes are queued
            for t, src in [(ST, st_in), (GI, gi_in), (S, s_in)]:
                nc.sync.dma_start(t[:], src[:])
            emit_ag(0, [0, 1, 2, 3])
            emit_ag(0, [4, 5, 6, 7])
            emit_ag(0, [8, 9])
            for s in range(SLOTS):
                ed_work(0, s)
            a3 = pp.tile([128, PER_CORE], BF16)
            gmpb = pp.tile([128, 8], BF16)
            gapb = pp.tile([128, 8], BF16)
            # ---- layers -------------------------------------------------
            # Explicit emission schedule: slots 0/1 interleave their tier
            # pieces at the layer boundary so the tail AllGathers (emitted
            # after the first t0 gathers) never block the GpSimd queue; t2
            # pieces + evict lag one slot behind; each next-layer AllGather
            # trigger is emitted ~2 slots after its producing tile so its
            # input wait is zero.
            T0, T1, T2, ALL = (4000,), (8000,), (N,), (4000, 8000, N)

            def nxt(l, s):
                if l < 2:
                    tile_work(l + 1, s)
                    ed_work(l + 1, s)

            for l in range(3):
                piece_work(l, 0, T0)
                piece_work(l, 1, T0)
                if l > 0:
                    emit_ag(l, [4, 5, 6, 7])
                    emit_ag(l, [8, 9])
                piece_work(l, 0, T1)
                piece_work(l, 1, T1)
                piece_work(l, 0, T2)
                evict_work(l, 0)
                nxt(l, 0)
                piece_work(l, 2, (4000, 8000))
                piece_work(l, 1, T2)
                evict_work(l, 1)
                nxt(l, 1)
                piece_work(l, 3, ALL)
                piece_work(l, 2, T2)
                evict_work(l, 2)
                nxt(l, 2)
                for s in range(4, SLOTS):
                    piece_work(l, s, ALL)
                    evict_work(l, s - 1)
                    nxt(l, s - 1)
                    if l < 2 and s == 7:
                        emit_ag(l + 1, [0, 1, 2, 3])
                evict_work(l, SLOTS - 1)
                nxt(l, SLOTS - 1)

            # ---- readout ------------------------------------------------
            phg = pscr.tile([128, 8], F32, tag="scr")
            nc.tensor.matmul(phg[:], L0W[:, 0, :], gmpb[:], start=True, stop=False)
            nc.tensor.matmul(phg[:], L0W[:, 1, :], gapb[:], start=False, stop=True)
            hg = pp.tile([128, 8], BF16)
            nc.scalar.activation(hg[:], phg[:], AF.Relu, bias=L0B[:])
            pnw = pscr.tile([128, 8], F32, tag="scr")
            nc.tensor.matmul(pnw[:], LNW[:, 0, :], XR[:, 0, :], start=True, stop=False)
            nc.tensor.matmul(pnw[:], LNW[:, 1, :], XR[:, 1, :], start=False, stop=True)
            nw = pp.tile([128, 8], BF16)
            nc.scalar.activation(nw[:], pnw[:], AF.Relu, bias=LNB[:])
            pfin = pscr.tile([8, 1], F32, tag="scr")
            nc.tensor.matmul(pfin[:], hg[:], L1W[:, 0, :], start=True, stop=False)
            nc.tensor.matmul(pfin[:], nw[:], L1W[:, 1, :], start=False, stop=True)
            fin = pp.tile([8, 1], F32)
            nc.scalar.activation(fin[:], pfin[:], AF.Sigmoid, bias=L1B[:])
            nc.sync.dma_start(out_t[:], fin[:])
    nc.compile()
    return nc


def kernel(x, edge_index, batch,
           W1, as1, ad1, b1, g1, bb1, m1, v1,
           W2, as2, ad2, b2, g2, bb2, m2, v2,
           W3, as3, ad3, b3, g3, bb3, m3, v3,
           lnW, lnb, l0W, l0b, l1W, l1b):
    x = np.asarray(x, np.float32)
    edge_index = np.asarray(edge_index)
    batch = np.asarray(batch)
    Ws = [np.asarray(w, np.float64) for w in (W1, W2, W3)]
    ass = [np.asarray(a, np.float64) for a in (as1, as2, as3)]
    ads = [np.asarray(a, np.float64) for a in (ad1, ad2, ad3)]
    bs = [np.asarray(a, np.float32) for a in (b1, b2, b3)]
    gs = [np.asarray(a, np.float32) for a in (g1, g2, g3)]
    bbs = [np.asarray(a, np.float32) for a in (bb1, bb2, bb3)]
    ms = [np.asarray(a, np.float32) for a in (m1, m2, m3)]
    vs = [np.asarray(a, np.float32) for a in (v1, v2, v3)]

    src = np.concatenate([edge_index[0], np.arange(N)]).astype(np.int64)
    dst = np.concatenate([edge_index[1], np.arange(N)]).astype(np.int64)
    tile_of = dst // TILE
    order = np.argsort(tile_of, kind="stable")
    src, dst, tile_of = src[order], dst[order], tile_of[order]
    bounds = np.searchsorted(tile_of, np.arange(81))
    cnts_ks = (bounds[1:] - bounds[:-1]).reshape(NCORES, SLOTS)
    nct = [int(np.ceil(cnts_ks[:, s] / 128).max()) for s in range(SLOTS)]
    nchunk = sum(nct)
    slot_c0 = np.concatenate([[0], np.cumsum(nct)]).astype(int)

    # grouped full-table row layout (same groups for every layer; layer 0
    # uses one big AG but the row layout only depends on AG_GROUPS for
    # consistency across layers -> use AG_GROUPS layout for all).
    full_row = np.zeros(N, np.int64)
    base = 0
    for grp in AG_GROUPS:
        gl = len(grp)
        for k in range(NCORES):
            for si, s in enumerate(grp):
                rows = np.arange(TILE) + k * PER_CORE + s * TILE
                full_row[rows] = base + k * TILE * gl + si * TILE + np.arange(TILE)
        base += gl * TILE * NCORES
    assert base == N

    gsrc = np.zeros((NCORES, nchunk * 128), np.int16)
    dloc = np.full((NCORES, nchunk * 128), -1, np.int64)
    tier_cnt = np.zeros((NCORES, SLOTS, 2), np.int64)
    for t in range(80):
        k, s = t // SLOTS, t % SLOTS
        lo, hi = bounds[t], bounds[t + 1]
        fr = full_row[src[lo:hi]]
        tier = (fr >= 4000).astype(np.int64) + (fr >= 8000)
        tord = np.argsort(tier, kind="stable")
        fr, dl_ = fr[tord], (dst[lo:hi] - t * TILE)[tord]
        tier_cnt[k, s, 0] = int((tier == 0).sum())
        tier_cnt[k, s, 1] = int((tier <= 1).sum())
        o = slot_c0[s] * 128
        gsrc[k, o:o + hi - lo] = fr
        dloc[k, o:o + hi - lo] = dl_
    # per-chunk src-row bound = min tier across cores (pad idx 0 is tier 0)
    chunk_lim = np.full(nchunk, N, np.int64)
    for s in range(SLOTS):
        for ci in range(nct[s]):
            e_end = (ci + 1) * 128
            if all(e_end <= tier_cnt[k, s, 0] for k in range(NCORES)):
                chunk_lim[slot_c0[s] + ci] = 4000
            elif all(e_end <= tier_cnt[k, s, 1] for k in range(NCORES)):
                chunk_lim[slot_c0[s] + ci] = 8000
    dl = dloc.reshape(NCORES, nchunk, 128)
    S_all = dl[:, :, :, None] == np.arange(128)[None, None, None, :]
    S_t = _f8(S_all.transpose(0, 2, 1, 3))   # [NCORES, 128e, nchunk, 128d]
    ST_t = _bf(S_all.transpose(0, 3, 1, 2)[:, 0:TILE])  # [NCORES, 125d, nchunk, 128e]
    gi = gsrc.reshape(NCORES, nchunk * 8, 16).transpose(0, 2, 1)
    gi = np.ascontiguousarray(np.tile(gi, (1, 8, 1)))

    wexts = []
    for li in range(3):
        W = Ws[li]
        ws = np.stack([W[:, h * C:(h + 1) * C] @ ass[li][h] for h in range(H)], 1)
        wd = np.stack([W[:, h * C:(h + 1) * C] @ ads[li][h] for h in range(H)], 1)
        wexts.append(np.concatenate([W, ws, wd], axis=1))
    w1 = _bf(wexts[0].reshape(2, 128, 1040).transpose(1, 0, 2))
    w2 = _bf(wexts[1][0:128])
    w3 = _bf(wexts[2][0:128])
    bn = np.zeros((128, 9, 128), np.float32)  # cast to bf16 below
    for li in range(3):
        r = 1.0 / np.sqrt(vs[li] + EPS)
        bn[:, 3 * li + 0] = 8.0 * bs[li]
        bn[:, 3 * li + 1] = gs[li] * r / 8.0
        bn[:, 3 * li + 2] = bbs[li] - ms[li] * gs[li] * r

    bnds = np.searchsorted(batch, np.arange(G + 1))
    assert bnds[G] == N
    for k in range(1, NCORES):
        assert bnds[8 * k] == PER_CORE * k, "graphs must align to cores"
    g0 = [int(bnds[g]) for g in range(9)]
    cnt = [float(bnds[g + 1] - bnds[g]) for g in range(8)]
    for k in range(1, NCORES):
        for g in range(9):
            assert int(bnds[8 * k + g]) - PER_CORE * k == g0[g]
    root = bnds[:G]
    xrT = np.asarray(x, np.float64)[root].T.reshape(2, 128, G)

    l0w = _bf(np.asarray(l0W).reshape(2, 128, 128).transpose(1, 0, 2))
    lnw = _bf(np.asarray(lnW).reshape(2, 128, 128).transpose(1, 0, 2))
    l1w = _bf(np.asarray(l1W).reshape(2, 128, 1).transpose(1, 0, 2))

    nc = build_nc(nct, [int(v) for v in chunk_lim], g0, cnt)

    x64 = np.asarray(x, np.float64)
    in_maps = []
    for k in range(NCORES):
        xk = x64[k * PER_CORE:(k + 1) * PER_CORE]     # [1250, 256]
        xTk = _bf(np.ascontiguousarray(
            xk.T.reshape(2, 128, PER_CORE).transpose(1, 0, 2)))
        in_maps.append(dict(
            xT=xTk,
            w1=w1, w2=w2, w3=w3,
            sfull=np.ascontiguousarray(S_t[k]),
            stfull=np.ascontiguousarray(ST_t[k]),
            gidx=gi[k],
            bn=_bf(bn), ident=_bf(np.eye(128)),
            xrootT=_bf(np.ascontiguousarray(
                xrT[:, :, 8 * k:8 * k + 8].transpose(1, 0, 2))),
            l0w=l0w, lnw=lnw, l1w=l1w,
            l0b=np.asarray(l0b, np.float32).reshape(128, 1),
            lnb=np.asarray(lnb, np.float32).reshape(128, 1),
            l1b=np.broadcast_to(np.asarray(l1b, np.float32), (8, 1)).copy(),
        ))
    global LAST_RESULT, LAST_NC, LAST_INMAPS
    LAST_NC, LAST_INMAPS = nc, in_maps
    res = run_bass_kernel_spmd(nc, in_maps, core_ids=list(range(NCORES)))
    LAST_RESULT = res
    out = np.concatenate([res.results[k]["out"] for k in range(NCORES)], 0)
    return out.astype(np.float32)
